# revision 1
# baseline (speedup 1.0000x reference)
"""Trainium2 Bass kernel for nn_MlpNet: Gaussian-window spectrogram + MLP.

Math:
  s[b, f, t] = |sum_n xc[b,n] * win[t,n] * e^{-2pi i f n / 2046}|^2
  out = relu(s.flat @ W1.T + b1) @ W2.T + b2

Sharding (8 cores): frequency bins f are split 128-per-core. Each core
computes its s[b, f_shard, t] slab via two f32r matmuls (cos/sin DFT
against the windowed signal), squares+adds on ACT/DVE, and accumulates
its fc1 partial h (bf16 matmuls against its column shard of W1, whose
feature index f*T+t is contiguous per f-shard). The host sums the 8
h-partials, applies b1/relu/fc2, and concatenates the s slabs.
"""
import os
import sys

sys.path.insert(0, "/opt/trn_rl_repo")

import numpy as np
import ml_dtypes

B = 16
N = 1023
T = 1024
F = 1024
HID = 128
NCLS = 10
NCORES = 8
FPC = F // NCORES       # 128 frequency bins per core
KT = 8                  # contraction tiles of 128 over n
TBS = 256               # t-block size (matmul free dim)
NTB = T // TBS          # 4 t-blocks
GS = 64                 # fc1 chunks per W1 DMA slab
SLABS = TBS // GS       # 4 slabs per t-block

LAST_RESULT = None


def _make_band(sigma):
    """Per t-block list of n-tiles (of 128) where the Gaussian window is
    non-negligible: |n - t| <= ceil(8 sigma) keeps every term above the
    fp32 noise floor of the sum."""
    w = int(np.ceil(8.0 * max(sigma, 1.0)))
    band = []
    for tb in range(NTB):
        lo = tb * TBS - w
        hi = tb * TBS + TBS - 1 + w
        ks = [k for k in range(KT) if 128 * k <= hi and 128 * k + 127 >= lo]
        band.append(ks)
    return band


def _build_program(band):
    import concourse.bacc as bacc
    import concourse.tile as tile
    from concourse import mybir

    f32 = mybir.dt.float32
    f32r = mybir.dt.float32r
    bf16 = mybir.dt.bfloat16
    Square = mybir.ActivationFunctionType.Square
    mult = mybir.AluOpType.mult

    nc = bacc.Bacc("TRN2", target_bir_lowering=False, debug=False,
                   num_devices=NCORES)

    xct_d = nc.dram_tensor("xct", [128, KT * B], f32, kind="ExternalInput")
    wint_d = nc.dram_tensor("wint", [128, KT * T], f32r, kind="ExternalInput")
    cs_d = nc.dram_tensor("cs", [128, KT * 2 * FPC], f32, kind="ExternalInput")
    w1h_d = nc.dram_tensor("w1h", [128, T * HID], bf16, kind="ExternalInput")
    s_out_d = nc.dram_tensor("s_out", [B, FPC, T], f32, kind="ExternalOutput")
    h_out_d = nc.dram_tensor("h_out", [B, HID], f32, kind="ExternalOutput")

    with tile.TileContext(nc) as tc:
        with (
            tc.tile_pool(name="const", bufs=1) as const_pool,
            tc.tile_pool(name="sbf", bufs=1) as sbf_pool,
            tc.tile_pool(name="csb", bufs=4) as csb_pool,
            tc.tile_pool(name="sq", bufs=4) as sq_pool,
            tc.tile_pool(name="stage", bufs=4) as stage_pool,
            tc.tile_pool(name="w1", bufs=3) as w1_pool,
            tc.tile_pool(name="hacc", bufs=1) as hacc_pool,
            tc.tile_pool(name="ps", bufs=2, space="PSUM") as ps_pool,
            tc.tile_pool(name="hps", bufs=2, space="PSUM") as hps_pool,
        ):
            xct_sb = const_pool.tile([128, KT * B], f32)
            wint_sb = const_pool.tile([128, KT * T], f32r)
            cs_sb = const_pool.tile([128, KT * 2 * FPC], f32)
            nc.sync.dma_start(xct_sb[:], xct_d[:])
            nc.sync.dma_start(wint_sb[:], wint_d[:])
            nc.sync.dma_start(cs_sb[:], cs_d[:])

            # bf16 copy of this core's s slab, laid out [f_p, b*T + t],
            # read back as the fc1 stationary operand.
            s_bf = sbf_pool.tile([128, B * T], bf16)
            s_bf_r = s_bf[:].rearrange("p (b t) -> p t b", b=B)

            h_acc = hacc_pool.tile([B, HID], f32)
            nc.vector.memset(h_acc[:], 0.0)

            for tb in range(NTB):
                t0 = tb * TBS
                ks = band[tb]
                for b in range(B):
                    csb = csb_pool.tile([128, KT * 2 * FPC], f32r)
                    for k in ks:
                        nc.vector.tensor_tensor(
                            csb[:, k * 256:(k + 1) * 256],
                            cs_sb[:, k * 256:(k + 1) * 256],
                            xct_sb[:, k * B + b:k * B + b + 1]
                            .broadcast_to([128, 256]),
                            mult,
                        )
                    ps_re = ps_pool.tile([128, TBS], f32, tag="psre")
                    ps_im = ps_pool.tile([128, TBS], f32, tag="psim")
                    for i, k in enumerate(ks):
                        rhs = wint_sb[:, k * T + t0:k * T + t0 + TBS]
                        nc.tensor.matmul(
                            ps_re[:], csb[:, k * 256:k * 256 + 128], rhs,
                            start=(i == 0), stop=(i == len(ks) - 1),
                        )
                        nc.tensor.matmul(
                            ps_im[:], csb[:, k * 256 + 128:(k + 1) * 256], rhs,
                            start=(i == 0), stop=(i == len(ks) - 1),
                        )
                    t_re = sq_pool.tile([128, TBS], f32, tag="tre")
                    t_im = sq_pool.tile([128, TBS], f32, tag="tim")
                    nc.scalar.activation(t_re[:], ps_re[:], Square)
                    nc.scalar.activation(t_im[:], ps_im[:], Square)
                    s_stage = stage_pool.tile([128, TBS], f32)
                    nc.vector.tensor_add(s_stage[:], t_re[:], t_im[:])
                    nc.sync.dma_start(s_out_d[b, :, t0:t0 + TBS], s_stage[:])
                    nc.vector.tensor_copy(
                        s_bf[:, b * T + t0:b * T + t0 + TBS], s_stage[:]
                    )

                # fc1 partial for the 256 t-chunks of this block
                h_ps = hps_pool.tile([B, HID], f32)
                for sl in range(SLABS):
                    c0 = (t0 + sl * GS) * HID
                    w1t = w1_pool.tile([128, GS * HID], bf16)
                    nc.sync.dma_start(w1t[:], w1h_d[:, c0:c0 + GS * HID])
                    for g in range(GS):
                        t = t0 + sl * GS + g
                        nc.tensor.matmul(
                            h_ps[:],
                            s_bf_r[:, t, :],
                            w1t[:, g * HID:(g + 1) * HID],
                            start=(sl == 0 and g == 0),
                            stop=(sl == SLABS - 1 and g == GS - 1),
                        )
                nc.vector.tensor_add(h_acc[:], h_acc[:], h_ps[:])

            nc.sync.dma_start(h_out_d[:], h_acc[:])

    nc.compile()
    return nc


def _host_precompute(x, lambd):
    sigma = float(abs(np.float32(np.asarray(lambd))))
    n_idx = np.arange(1024, dtype=np.float64)[:, None]   # padded n (1023 -> 0)
    t_idx = np.arange(T, dtype=np.float64)[None, :]

    xc = (x - x.mean(axis=-1, keepdims=True)).astype(np.float32)  # [B, N]
    xct = np.zeros((1024, B), np.float32)
    xct[:N] = xc.T
    xct = xct.reshape(KT, 128, B).transpose(1, 0, 2).reshape(128, KT * B)
    xct = np.ascontiguousarray(xct)

    winT = np.exp(-0.5 * ((n_idx - t_idx) / sigma) ** 2).astype(np.float32)
    winT[N:] = 0.0
    wint = winT.reshape(KT, 128, T).transpose(1, 0, 2).reshape(128, KT * T)
    wint = np.ascontiguousarray(wint)

    cs_list = []
    for c in range(NCORES):
        f_idx = (c * FPC + np.arange(FPC, dtype=np.float64))[None, :]
        ang = 2.0 * np.pi * n_idx * f_idx / (2.0 * N)
        cs = np.concatenate(
            [np.cos(ang), np.sin(ang)], axis=1
        ).astype(np.float32)                                  # [1024, 256]
        cs[N:] = 0.0
        cs = cs.reshape(KT, 128, 2 * FPC).transpose(1, 0, 2)
        cs_list.append(np.ascontiguousarray(cs.reshape(128, KT * 2 * FPC)))

    return sigma, xct, wint, cs_list


def _transpose_w1(W1):
    """Blocked transpose W1 [HID, F*T] -> [F*T, HID] in bf16."""
    feat = W1.shape[1]
    W1T = np.empty((feat, HID), ml_dtypes.bfloat16)
    bs = 8192
    for i in range(0, feat, bs):
        W1T[i:i + bs] = W1[:, i:i + bs].T.astype(ml_dtypes.bfloat16)
    return W1T


def kernel(x, lambd, W1, b1, W2, b2):
    global LAST_RESULT
    from concourse.bass_utils import run_bass_kernel_spmd

    x = np.asarray(x, np.float32)
    W1 = np.asarray(W1, np.float32)
    b1 = np.asarray(b1, np.float32)
    W2 = np.asarray(W2, np.float32)
    b2 = np.asarray(b2, np.float32)

    sigma, xct, wint, cs_list = _host_precompute(x, lambd)
    band = _make_band(sigma)

    W1T = _transpose_w1(W1)                     # [F*T, HID] bf16
    w1h = W1T.reshape(NCORES, 128, T * HID)     # per-core contiguous views

    nc = _build_program(band)

    in_maps = [
        dict(xct=xct, wint=wint, cs=cs_list[c], w1h=np.ascontiguousarray(w1h[c]))
        for c in range(NCORES)
    ]
    res = run_bass_kernel_spmd(
        nc, in_maps, list(range(NCORES)),
        trace=bool(os.environ.get("KERNEL_TRACE")),
    )
    LAST_RESULT = res

    s = np.concatenate(
        [res.results[c]["s_out"] for c in range(NCORES)], axis=1
    )                                            # [B, F, T]
    h = np.zeros((B, HID), np.float32)
    for c in range(NCORES):
        h += res.results[c]["h_out"]
    h = np.maximum(h + b1, 0.0)
    out = h @ W2.T + b2
    return out.astype(np.float32), s[:, None].astype(np.float32)


# revision 2
# speedup vs baseline: 1.1156x; 1.1156x over previous
"""Trainium2 Bass kernel for nn_MlpNet: Gaussian-window spectrogram + MLP.

Math:
  s[b, f, t] = |sum_n xc[b,n] * win[t,n] * e^{-2pi i f n / 2046}|^2
  out = relu(s.flat @ W1.T + b1) @ W2.T + b2

Sharding (8 cores): frequency bins f are split 128-per-core. Each core
computes its s[b, f_shard, t] slab via two f32r matmuls (cos/sin DFT
against the windowed signal), squares+adds on ACT/DVE, and accumulates
its fc1 partial h (bf16 matmuls against its column shard of W1, whose
feature index f*T+t is contiguous per f-shard). The host sums the 8
h-partials, applies b1/relu/fc2, and concatenates the s slabs.

The Gaussian window is truncated at |n - t| > 8 sigma (below the fp32
noise floor of the 1023-term sum), which skips ~40% of the DFT matmuls.
"""
import os
import sys

sys.path.insert(0, "/opt/trn_rl_repo")

import numpy as np
import ml_dtypes

B = 16
N = 1023
T = 1024
F = 1024
HID = 128
NCLS = 10
NCORES = 8
FPC = F // NCORES       # 128 frequency bins per core
KT = 8                  # contraction tiles of 128 over n
TBS = 256               # t-block size (matmul free dim)
NTB = T // TBS          # 4 t-blocks
GS = 64                 # fc1 chunks per W1 DMA slab
NSLAB = T // GS         # 16 slabs

LAST_RESULT = None


def _make_band(sigma):
    """Per t-block list of n-tiles (of 128) where the Gaussian window is
    non-negligible: |n - t| <= ceil(8 sigma) keeps every term above the
    fp32 noise floor of the sum."""
    w = int(np.ceil(8.0 * max(sigma, 1.0)))
    band = []
    for tb in range(NTB):
        lo = tb * TBS - w
        hi = tb * TBS + TBS - 1 + w
        ks = [k for k in range(KT) if 128 * k <= hi and 128 * k + 127 >= lo]
        band.append(ks)
    return band


def _build_program(band):
    import concourse.bacc as bacc
    import concourse.tile as tile
    from concourse import mybir

    f32 = mybir.dt.float32
    f32r = mybir.dt.float32r
    bf16 = mybir.dt.bfloat16
    Square = mybir.ActivationFunctionType.Square

    nc = bacc.Bacc("TRN2", target_bir_lowering=False, debug=False,
                   num_devices=NCORES)

    xct_d = nc.dram_tensor("xct", [128, KT * B], f32, kind="ExternalInput")
    wint_d = nc.dram_tensor("wint", [128, KT * T], f32r, kind="ExternalInput")
    cs_d = nc.dram_tensor("cs", [128, KT * 2 * FPC], f32, kind="ExternalInput")
    w1h_d = nc.dram_tensor("w1h", [128, T * HID], bf16, kind="ExternalInput")
    s_out_d = nc.dram_tensor("s_out", [B, FPC, T], f32, kind="ExternalOutput")
    h_out_d = nc.dram_tensor("h_out", [B, HID], f32, kind="ExternalOutput")

    with tile.TileContext(nc) as tc:
        with (
            tc.tile_pool(name="const", bufs=1) as const_pool,
            tc.tile_pool(name="sbf", bufs=1) as sbf_pool,
            tc.tile_pool(name="csb", bufs=2) as csb_pool,
            tc.tile_pool(name="sq", bufs=4) as sq_pool,
            tc.tile_pool(name="stage", bufs=2) as stage_pool,
            tc.tile_pool(name="w1", bufs=5) as w1_pool,
            tc.tile_pool(name="hacc", bufs=1) as hacc_pool,
            tc.tile_pool(name="ps", bufs=2, space="PSUM") as ps_pool,
            tc.tile_pool(name="hps", bufs=1, space="PSUM") as hps_pool,
        ):
            xct_sb = const_pool.tile([128, KT * B], f32)
            wint_sb = const_pool.tile([128, KT * T], f32r)
            cs_sb = const_pool.tile([128, KT * 2 * FPC], f32)
            nc.sync.dma_start(xct_sb[:], xct_d[:])
            nc.sync.dma_start(wint_sb[:], wint_d[:])
            nc.sync.dma_start(cs_sb[:], cs_d[:])

            # bf16 copy of this core's s slab, laid out [f_p, b*T + t],
            # read back as the fc1 stationary operand.
            s_bf = sbf_pool.tile([128, B * T], bf16)
            s_bf_r = s_bf[:].rearrange("p (b t) -> p t b", b=B)

            # ---- spectrogram: per-sample DFT of the windowed signal ----
            for b in range(B):
                csb = csb_pool.tile([128, KT * 2 * FPC], f32r)
                for k in range(KT):
                    nc.vector.tensor_scalar_mul(
                        csb[:, k * 256:(k + 1) * 256],
                        cs_sb[:, k * 256:(k + 1) * 256],
                        xct_sb[:, k * B + b:k * B + b + 1],
                    )
                s_stage = stage_pool.tile([128, T], f32)
                for tb in range(NTB):
                    t0 = tb * TBS
                    ks = band[tb]
                    ps_re = ps_pool.tile([128, TBS], f32, tag="psre")
                    ps_im = ps_pool.tile([128, TBS], f32, tag="psim")
                    for i, k in enumerate(ks):
                        rhs = wint_sb[:, k * T + t0:k * T + t0 + TBS]
                        nc.tensor.matmul(
                            ps_re[:], csb[:, k * 256:k * 256 + 128], rhs,
                            start=(i == 0), stop=(i == len(ks) - 1),
                        )
                        nc.tensor.matmul(
                            ps_im[:], csb[:, k * 256 + 128:(k + 1) * 256], rhs,
                            start=(i == 0), stop=(i == len(ks) - 1),
                        )
                    t_re = sq_pool.tile([128, TBS], f32, tag="tre")
                    t_im = sq_pool.tile([128, TBS], f32, tag="tim")
                    nc.scalar.activation(t_re[:], ps_re[:], Square)
                    nc.scalar.activation(t_im[:], ps_im[:], Square)
                    nc.vector.tensor_add(
                        s_stage[:, t0:t0 + TBS], t_re[:], t_im[:]
                    )
                    nc.vector.tensor_copy(
                        s_bf[:, b * T + t0:b * T + t0 + TBS],
                        s_stage[:, t0:t0 + TBS],
                    )
                nc.sync.dma_start(s_out_d[b], s_stage[:])

            # ---- fc1 partial: h[b,:] += s[b,f,t] * W1[:, f*T+t] ----
            h_ps = hps_pool.tile([B, HID], f32)
            for sl in range(NSLAB):
                c0 = sl * GS * HID
                w1t = w1_pool.tile([128, GS * HID], bf16)
                nc.scalar.dma_start(w1t[:], w1h_d[:, c0:c0 + GS * HID])
                for g in range(GS):
                    t = sl * GS + g
                    nc.tensor.matmul(
                        h_ps[:],
                        s_bf_r[:, t, :],
                        w1t[:, g * HID:(g + 1) * HID],
                        start=(t == 0),
                        stop=(t == T - 1),
                    )
            h_acc = hacc_pool.tile([B, HID], f32)
            nc.vector.tensor_copy(h_acc[:], h_ps[:])
            nc.sync.dma_start(h_out_d[:], h_acc[:])

    nc.compile()
    return nc


def _host_precompute(x, lambd):
    sigma = float(abs(np.float32(np.asarray(lambd))))
    n_idx = np.arange(1024, dtype=np.float64)[:, None]   # padded n (1023 -> 0)
    t_idx = np.arange(T, dtype=np.float64)[None, :]

    xc = (x - x.mean(axis=-1, keepdims=True)).astype(np.float32)  # [B, N]
    xct = np.zeros((1024, B), np.float32)
    xct[:N] = xc.T
    xct = xct.reshape(KT, 128, B).transpose(1, 0, 2).reshape(128, KT * B)
    xct = np.ascontiguousarray(xct)

    winT = np.exp(-0.5 * ((n_idx - t_idx) / sigma) ** 2).astype(np.float32)
    winT[N:] = 0.0
    wint = winT.reshape(KT, 128, T).transpose(1, 0, 2).reshape(128, KT * T)
    wint = np.ascontiguousarray(wint)

    cs_list = []
    for c in range(NCORES):
        f_idx = (c * FPC + np.arange(FPC, dtype=np.float64))[None, :]
        ang = 2.0 * np.pi * n_idx * f_idx / (2.0 * N)
        cs = np.concatenate(
            [np.cos(ang), np.sin(ang)], axis=1
        ).astype(np.float32)                                  # [1024, 256]
        cs[N:] = 0.0
        cs = cs.reshape(KT, 128, 2 * FPC).transpose(1, 0, 2)
        cs_list.append(np.ascontiguousarray(cs.reshape(128, KT * 2 * FPC)))

    return sigma, xct, wint, cs_list


def _transpose_w1(W1):
    """Blocked transpose W1 [HID, F*T] -> [F*T, HID] in bf16."""
    feat = W1.shape[1]
    W1T = np.empty((feat, HID), ml_dtypes.bfloat16)
    bs = 8192
    for i in range(0, feat, bs):
        W1T[i:i + bs] = W1[:, i:i + bs].T.astype(ml_dtypes.bfloat16)
    return W1T


def kernel(x, lambd, W1, b1, W2, b2):
    global LAST_RESULT
    from concourse.bass_utils import run_bass_kernel_spmd

    x = np.asarray(x, np.float32)
    W1 = np.asarray(W1, np.float32)
    b1 = np.asarray(b1, np.float32)
    W2 = np.asarray(W2, np.float32)
    b2 = np.asarray(b2, np.float32)

    sigma, xct, wint, cs_list = _host_precompute(x, lambd)
    band = _make_band(sigma)

    W1T = _transpose_w1(W1)                     # [F*T, HID] bf16
    w1h = W1T.reshape(NCORES, 128, T * HID)     # per-core contiguous views

    nc = _build_program(band)

    in_maps = [
        dict(xct=xct, wint=wint, cs=cs_list[c], w1h=np.ascontiguousarray(w1h[c]))
        for c in range(NCORES)
    ]
    res = run_bass_kernel_spmd(
        nc, in_maps, list(range(NCORES)),
        trace=bool(os.environ.get("KERNEL_TRACE")),
    )
    LAST_RESULT = res

    s = np.concatenate(
        [res.results[c]["s_out"] for c in range(NCORES)], axis=1
    )                                            # [B, F, T]
    h = np.zeros((B, HID), np.float32)
    for c in range(NCORES):
        h += res.results[c]["h_out"]
    h = np.maximum(h + b1, 0.0)
    out = h @ W2.T + b2
    return out.astype(np.float32), s[:, None].astype(np.float32)


# revision 4
# speedup vs baseline: 1.2242x; 1.0974x over previous
"""Trainium2 Bass kernel for nn_MlpNet: Gaussian-window spectrogram + MLP.

Math:
  s[b, f, t] = |sum_n xc[b,n] * win[t,n] * e^{-2pi i f n / 2046}|^2
  out = relu(s.flat @ W1.T + b1) @ W2.T + b2

Sharding (8 cores): frequency bins f are split 128-per-core. Each core
computes its s[b, f_shard, t] slab via two f32r matmuls (cos/sin DFT
against the windowed signal), squares+adds on ACT/DVE, and accumulates
its fc1 partial h (bf16 matmuls against its column shard of W1, whose
feature index f*T+t is contiguous per f-shard). The host sums the 8
h-partials, applies b1/relu/fc2, and concatenates the s slabs.

The Gaussian window is truncated at |n - t| > 8 sigma (below the fp32
noise floor of the 1023-term sum), which skips ~40% of the DFT matmuls.
"""
import os
import sys

sys.path.insert(0, "/opt/trn_rl_repo")

import numpy as np
import ml_dtypes

B = 16
N = 1023
T = 1024
F = 1024
HID = 128
NCLS = 10
NCORES = 8
FPC = F // NCORES       # 128 frequency bins per core
KT = 8                  # contraction tiles of 128 over n
TBS = 256               # t-block size (matmul free dim)
NTB = T // TBS          # 4 t-blocks
GS = 64                 # fc1 chunks per W1 DMA slab
NSLAB = T // GS         # 16 slabs

LAST_RESULT = None


def _make_band(sigma):
    """Per t-block list of n-tiles (of 128) where the Gaussian window is
    non-negligible: |n - t| <= ceil(8 sigma) keeps every term above the
    fp32 noise floor of the sum."""
    w = int(np.ceil(8.0 * max(sigma, 1.0)))
    band = []
    for tb in range(NTB):
        lo = tb * TBS - w
        hi = tb * TBS + TBS - 1 + w
        ks = [k for k in range(KT) if 128 * k <= hi and 128 * k + 127 >= lo]
        band.append(ks)
    return band


def _build_program(band):
    import concourse.bacc as bacc
    import concourse.tile as tile
    from concourse import mybir

    f32 = mybir.dt.float32
    f32r = mybir.dt.float32r
    bf16 = mybir.dt.bfloat16
    Square = mybir.ActivationFunctionType.Square

    nc = bacc.Bacc("TRN2", target_bir_lowering=False, debug=False,
                   num_devices=NCORES)

    xct_d = nc.dram_tensor("xct", [128, KT * B], f32, kind="ExternalInput")
    wint_d = nc.dram_tensor("wint", [128, KT * T], f32r, kind="ExternalInput")
    cs_d = nc.dram_tensor("cs", [128, KT * 2 * FPC], f32, kind="ExternalInput")
    w1h_d = nc.dram_tensor("w1h", [128, T * HID], bf16, kind="ExternalInput")
    s_out_d = nc.dram_tensor("s_out", [B, FPC, T], f32, kind="ExternalOutput")
    h_out_d = nc.dram_tensor("h_out", [B, HID], f32, kind="ExternalOutput")

    with tile.TileContext(nc) as tc:
        with (
            tc.tile_pool(name="const", bufs=1) as const_pool,
            tc.tile_pool(name="sbf", bufs=1) as sbf_pool,
            tc.tile_pool(name="csb", bufs=2) as csb_pool,
            tc.tile_pool(name="sq", bufs=4) as sq_pool,
            tc.tile_pool(name="stage", bufs=2) as stage_pool,
            tc.tile_pool(name="w1", bufs=5) as w1_pool,
            tc.tile_pool(name="hacc", bufs=1) as hacc_pool,
            tc.tile_pool(name="ps", bufs=2, space="PSUM") as ps_pool,
            tc.tile_pool(name="hps", bufs=1, space="PSUM") as hps_pool,
        ):
            # All input loads go on the sync HWDGE ring: FIFO order
            # guarantees the small consts land before the W1 stream starts.
            xct_sb = const_pool.tile([128, KT * B], f32)
            wint_sb = const_pool.tile([128, KT * T], f32r)
            cs_sb = const_pool.tile([128, KT * 2 * FPC], f32)
            nc.sync.dma_start(xct_sb[:], xct_d[:])
            nc.sync.dma_start(cs_sb[:], cs_d[:])
            for k in range(KT):
                nc.sync.dma_start(
                    wint_sb[:, k * T:(k + 1) * T], wint_d[:, k * T:(k + 1) * T]
                )

            # bf16 copy of this core's s slab, laid out [f_p, b*T + t],
            # read back as the fc1 stationary operand.
            s_bf = sbf_pool.tile([128, B * T], bf16)
            s_bf_r = s_bf[:].rearrange("p (b t) -> p t b", b=B)

            # ---- spectrogram: per-sample DFT of the windowed signal ----
            for b in range(B):
                csb = csb_pool.tile([128, KT * 2 * FPC], f32r)
                for k in range(KT):
                    nc.vector.tensor_scalar_mul(
                        csb[:, k * 256:(k + 1) * 256],
                        cs_sb[:, k * 256:(k + 1) * 256],
                        xct_sb[:, k * B + b:k * B + b + 1],
                    )
                s_stage = stage_pool.tile([128, T], f32)
                for tb in range(NTB):
                    t0 = tb * TBS
                    ks = band[tb]
                    ps_re = ps_pool.tile([128, TBS], f32, tag="psre")
                    ps_im = ps_pool.tile([128, TBS], f32, tag="psim")
                    for i, k in enumerate(ks):
                        rhs = wint_sb[:, k * T + t0:k * T + t0 + TBS]
                        nc.tensor.matmul(
                            ps_re[:], csb[:, k * 256:k * 256 + 128], rhs,
                            start=(i == 0), stop=(i == len(ks) - 1),
                        )
                        nc.tensor.matmul(
                            ps_im[:], csb[:, k * 256 + 128:(k + 1) * 256], rhs,
                            start=(i == 0), stop=(i == len(ks) - 1),
                        )
                    t_re = sq_pool.tile([128, TBS], f32, tag="tre")
                    t_im = sq_pool.tile([128, TBS], f32, tag="tim")
                    nc.scalar.activation(t_re[:], ps_re[:], Square)
                    nc.scalar.activation(t_im[:], ps_im[:], Square)
                    nc.vector.tensor_add(
                        s_stage[:, t0:t0 + TBS], t_re[:], t_im[:]
                    )
                    nc.vector.tensor_copy(
                        s_bf[:, b * T + t0:b * T + t0 + TBS],
                        s_stage[:, t0:t0 + TBS],
                    )
                nc.gpsimd.dma_start(s_out_d[b], s_stage[:])

            # ---- fc1 partial: h[b,:] += s[b,f,t] * W1[:, f*T+t] ----
            h_ps = hps_pool.tile([B, HID], f32)
            for sl in range(NSLAB):
                c0 = sl * GS * HID
                w1t = w1_pool.tile([128, GS * HID], bf16)
                nc.sync.dma_start(w1t[:], w1h_d[:, c0:c0 + GS * HID])
                for g in range(GS):
                    t = sl * GS + g
                    nc.tensor.matmul(
                        h_ps[:],
                        s_bf_r[:, t, :],
                        w1t[:, g * HID:(g + 1) * HID],
                        start=(t == 0),
                        stop=(t == T - 1),
                    )
            h_acc = hacc_pool.tile([B, HID], f32)
            nc.vector.tensor_copy(h_acc[:], h_ps[:])
            nc.gpsimd.dma_start(h_out_d[:], h_acc[:])

    nc.compile()
    return nc


def _host_precompute(x, lambd):
    sigma = float(abs(np.float32(np.asarray(lambd))))
    n_idx = np.arange(1024, dtype=np.float64)[:, None]   # padded n (1023 -> 0)
    t_idx = np.arange(T, dtype=np.float64)[None, :]

    xc = (x - x.mean(axis=-1, keepdims=True)).astype(np.float32)  # [B, N]
    xct = np.zeros((1024, B), np.float32)
    xct[:N] = xc.T
    xct = xct.reshape(KT, 128, B).transpose(1, 0, 2).reshape(128, KT * B)
    xct = np.ascontiguousarray(xct)

    winT = np.exp(-0.5 * ((n_idx - t_idx) / sigma) ** 2).astype(np.float32)
    winT[N:] = 0.0
    wint = winT.reshape(KT, 128, T).transpose(1, 0, 2).reshape(128, KT * T)
    wint = np.ascontiguousarray(wint)

    cs_list = []
    for c in range(NCORES):
        f_idx = (c * FPC + np.arange(FPC, dtype=np.float64))[None, :]
        ang = 2.0 * np.pi * n_idx * f_idx / (2.0 * N)
        cs = np.concatenate(
            [np.cos(ang), np.sin(ang)], axis=1
        ).astype(np.float32)                                  # [1024, 256]
        cs[N:] = 0.0
        cs = cs.reshape(KT, 128, 2 * FPC).transpose(1, 0, 2)
        cs_list.append(np.ascontiguousarray(cs.reshape(128, KT * 2 * FPC)))

    return sigma, xct, wint, cs_list


def _transpose_w1(W1):
    """Blocked transpose W1 [HID, F*T] -> [F*T, HID] in bf16."""
    feat = W1.shape[1]
    W1T = np.empty((feat, HID), ml_dtypes.bfloat16)
    bs = 8192
    for i in range(0, feat, bs):
        W1T[i:i + bs] = W1[:, i:i + bs].T.astype(ml_dtypes.bfloat16)
    return W1T


def kernel(x, lambd, W1, b1, W2, b2):
    global LAST_RESULT
    from concourse.bass_utils import run_bass_kernel_spmd

    x = np.asarray(x, np.float32)
    W1 = np.asarray(W1, np.float32)
    b1 = np.asarray(b1, np.float32)
    W2 = np.asarray(W2, np.float32)
    b2 = np.asarray(b2, np.float32)

    sigma, xct, wint, cs_list = _host_precompute(x, lambd)
    band = _make_band(sigma)

    W1T = _transpose_w1(W1)                     # [F*T, HID] bf16
    w1h = W1T.reshape(NCORES, 128, T * HID)     # per-core contiguous views

    nc = _build_program(band)

    in_maps = [
        dict(xct=xct, wint=wint, cs=cs_list[c], w1h=np.ascontiguousarray(w1h[c]))
        for c in range(NCORES)
    ]
    res = run_bass_kernel_spmd(
        nc, in_maps, list(range(NCORES)),
        trace=bool(os.environ.get("KERNEL_TRACE")),
    )
    LAST_RESULT = res

    s = np.concatenate(
        [res.results[c]["s_out"] for c in range(NCORES)], axis=1
    )                                            # [B, F, T]
    h = np.zeros((B, HID), np.float32)
    for c in range(NCORES):
        h += res.results[c]["h_out"]
    h = np.maximum(h + b1, 0.0)
    out = h @ W2.T + b2
    return out.astype(np.float32), s[:, None].astype(np.float32)


# revision 5
# speedup vs baseline: 1.2294x; 1.0043x over previous
"""Trainium2 Bass kernel for nn_MlpNet: Gaussian-window spectrogram + MLP.

Math:
  s[b, f, t] = |sum_n xc[b,n] * win[t,n] * e^{-2pi i f n / 2046}|^2
  out = relu(s.flat @ W1.T + b1) @ W2.T + b2

Sharding (8 cores): frequency bins f are split 128-per-core. Each core
computes its s[b, f_shard, t] slab via two f32r matmuls (cos/sin DFT
against the windowed signal), squares+adds on ACT/DVE, and accumulates
its fc1 partial h (bf16 matmuls against its column shard of W1, whose
feature index f*T+t is contiguous per f-shard). The host sums the 8
h-partials, applies b1/relu/fc2, and concatenates the s slabs.

The Gaussian window is truncated at |n - t| > 8 sigma (below the fp32
noise floor of the 1023-term sum), which skips ~40% of the DFT matmuls.
"""
import os
import sys

sys.path.insert(0, "/opt/trn_rl_repo")

import numpy as np
import ml_dtypes

B = 16
N = 1023
T = 1024
F = 1024
HID = 128
NCLS = 10
NCORES = 8
FPC = F // NCORES       # 128 frequency bins per core
KT = 8                  # contraction tiles of 128 over n
TBS = 256               # t-block size (matmul free dim)
NTB = T // TBS          # 4 t-blocks
GS = 64                 # fc1 chunks per W1 DMA slab
NSLAB = T // GS         # 16 slabs

LAST_RESULT = None


def _make_blocks(band):
    """Flatten the (tb, k) band structure into a packed block list;
    returns (blocks, index) where blocks[i] = (k, tb) and
    index[(k, tb)] = i."""
    blocks = []
    index = {}
    for tb in range(NTB):
        for k in band[tb]:
            index[(k, tb)] = len(blocks)
            blocks.append((k, tb))
    return blocks, index


def _make_band(sigma):
    """Per t-block list of n-tiles (of 128) where the Gaussian window is
    non-negligible: |n - t| <= ceil(8 sigma) keeps every term above the
    fp32 noise floor of the sum."""
    w = int(np.ceil(8.0 * max(sigma, 1.0)))
    band = []
    for tb in range(NTB):
        lo = tb * TBS - w
        hi = tb * TBS + TBS - 1 + w
        ks = [k for k in range(KT) if 128 * k <= hi and 128 * k + 127 >= lo]
        band.append(ks)
    return band


def _build_program(band, nblk):
    import concourse.bacc as bacc
    import concourse.tile as tile
    from concourse import mybir

    f32 = mybir.dt.float32
    f32r = mybir.dt.float32r
    bf16 = mybir.dt.bfloat16
    Square = mybir.ActivationFunctionType.Square

    nc = bacc.Bacc("TRN2", target_bir_lowering=False, debug=False,
                   num_devices=NCORES)

    xct_d = nc.dram_tensor("xct", [128, KT * B], f32, kind="ExternalInput")
    wint_d = nc.dram_tensor("wint", [128, nblk * TBS], f32r, kind="ExternalInput")
    cs_d = nc.dram_tensor("cs", [128, KT * 2 * FPC], f32, kind="ExternalInput")
    w1h_d = nc.dram_tensor("w1h", [128, T * HID], bf16, kind="ExternalInput")
    s_out_d = nc.dram_tensor("s_out", [B, FPC, T], f32, kind="ExternalOutput")
    h_out_d = nc.dram_tensor("h_out", [B, HID], f32, kind="ExternalOutput")

    blocks, blkidx = _make_blocks(band)
    with tile.TileContext(nc) as tc:
        with (
            tc.tile_pool(name="const", bufs=1) as const_pool,
            tc.tile_pool(name="sbf", bufs=1) as sbf_pool,
            tc.tile_pool(name="csb", bufs=2) as csb_pool,
            tc.tile_pool(name="sq", bufs=4) as sq_pool,
            tc.tile_pool(name="stage", bufs=2) as stage_pool,
            tc.tile_pool(name="w1", bufs=6) as w1_pool,
            tc.tile_pool(name="hacc", bufs=1) as hacc_pool,
            tc.tile_pool(name="ps", bufs=2, space="PSUM") as ps_pool,
            tc.tile_pool(name="hps", bufs=1, space="PSUM") as hps_pool,
        ):
            # All input loads go on the sync HWDGE ring: FIFO order
            # guarantees the small consts land before the W1 stream starts.
            xct_sb = const_pool.tile([128, KT * B], f32)
            wint_sb = const_pool.tile([128, nblk * TBS], f32r)
            cs_sb = const_pool.tile([128, KT * 2 * FPC], f32)
            nc.sync.dma_start(xct_sb[:], xct_d[:])
            nc.sync.dma_start(cs_sb[:], cs_d[:])
            half = (nblk // 2) * TBS
            nc.sync.dma_start(wint_sb[:, :half], wint_d[:, :half])
            nc.sync.dma_start(wint_sb[:, half:], wint_d[:, half:])

            # bf16 copy of this core's s slab, laid out [f_p, b*T + t],
            # read back as the fc1 stationary operand.
            s_bf = sbf_pool.tile([128, B * T], bf16)
            s_bf_r = s_bf[:].rearrange("p (b t) -> p t b", b=B)

            # ---- spectrogram: per-sample DFT of the windowed signal ----
            for b in range(B):
                csb = csb_pool.tile([128, KT * 2 * FPC], f32r)
                for k in range(KT):
                    nc.vector.tensor_scalar_mul(
                        csb[:, k * 256:(k + 1) * 256],
                        cs_sb[:, k * 256:(k + 1) * 256],
                        xct_sb[:, k * B + b:k * B + b + 1],
                    )
                s_stage = stage_pool.tile([128, T], f32)
                for tb in range(NTB):
                    t0 = tb * TBS
                    ks = band[tb]
                    ps_re = ps_pool.tile([128, TBS], f32, tag="psre")
                    ps_im = ps_pool.tile([128, TBS], f32, tag="psim")
                    for i, k in enumerate(ks):
                        blk = blkidx[(k, tb)]
                        rhs = wint_sb[:, blk * TBS:(blk + 1) * TBS]
                        nc.tensor.matmul(
                            ps_re[:], csb[:, k * 256:k * 256 + 128], rhs,
                            start=(i == 0), stop=(i == len(ks) - 1),
                        )
                        nc.tensor.matmul(
                            ps_im[:], csb[:, k * 256 + 128:(k + 1) * 256], rhs,
                            start=(i == 0), stop=(i == len(ks) - 1),
                        )
                    t_re = sq_pool.tile([128, TBS], f32, tag="tre")
                    t_im = sq_pool.tile([128, TBS], f32, tag="tim")
                    nc.scalar.activation(t_re[:], ps_re[:], Square)
                    nc.scalar.activation(t_im[:], ps_im[:], Square)
                    nc.vector.tensor_add(
                        s_stage[:, t0:t0 + TBS], t_re[:], t_im[:]
                    )
                    nc.vector.tensor_copy(
                        s_bf[:, b * T + t0:b * T + t0 + TBS],
                        s_stage[:, t0:t0 + TBS],
                    )
                nc.gpsimd.dma_start(s_out_d[b], s_stage[:])

            # ---- fc1 partial: h[b,:] += s[b,f,t] * W1[:, f*T+t] ----
            h_ps = hps_pool.tile([B, HID], f32)
            for sl in range(NSLAB):
                c0 = sl * GS * HID
                w1t = w1_pool.tile([128, GS * HID], bf16)
                nc.sync.dma_start(w1t[:], w1h_d[:, c0:c0 + GS * HID])
                for g in range(GS):
                    t = sl * GS + g
                    nc.tensor.matmul(
                        h_ps[:],
                        s_bf_r[:, t, :],
                        w1t[:, g * HID:(g + 1) * HID],
                        start=(t == 0),
                        stop=(t == T - 1),
                    )
            h_acc = hacc_pool.tile([B, HID], f32)
            nc.vector.tensor_copy(h_acc[:], h_ps[:])
            nc.gpsimd.dma_start(h_out_d[:], h_acc[:])

    nc.compile()
    return nc


def _host_precompute(x, lambd):
    sigma = float(abs(np.float32(np.asarray(lambd))))
    n_idx = np.arange(1024, dtype=np.float64)[:, None]   # padded n (1023 -> 0)
    t_idx = np.arange(T, dtype=np.float64)[None, :]

    xc = (x - x.mean(axis=-1, keepdims=True)).astype(np.float32)  # [B, N]
    xct = np.zeros((1024, B), np.float32)
    xct[:N] = xc.T
    xct = xct.reshape(KT, 128, B).transpose(1, 0, 2).reshape(128, KT * B)
    xct = np.ascontiguousarray(xct)

    winT = np.exp(-0.5 * ((n_idx - t_idx) / sigma) ** 2).astype(np.float32)
    winT[N:] = 0.0
    wfull = winT.reshape(KT, 128, T)      # [k, p, t]

    cs_list = []
    for c in range(NCORES):
        f_idx = (c * FPC + np.arange(FPC, dtype=np.float64))[None, :]
        ang = 2.0 * np.pi * n_idx * f_idx / (2.0 * N)
        cs = np.concatenate(
            [np.cos(ang), np.sin(ang)], axis=1
        ).astype(np.float32)                                  # [1024, 256]
        cs[N:] = 0.0
        cs = cs.reshape(KT, 128, 2 * FPC).transpose(1, 0, 2)
        cs_list.append(np.ascontiguousarray(cs.reshape(128, KT * 2 * FPC)))

    return sigma, xct, wfull, cs_list


def _transpose_w1(W1):
    """Blocked transpose W1 [HID, F*T] -> [F*T, HID] in bf16."""
    feat = W1.shape[1]
    W1T = np.empty((feat, HID), ml_dtypes.bfloat16)
    bs = 8192
    for i in range(0, feat, bs):
        W1T[i:i + bs] = W1[:, i:i + bs].T.astype(ml_dtypes.bfloat16)
    return W1T


def kernel(x, lambd, W1, b1, W2, b2):
    global LAST_RESULT
    from concourse.bass_utils import run_bass_kernel_spmd

    x = np.asarray(x, np.float32)
    W1 = np.asarray(W1, np.float32)
    b1 = np.asarray(b1, np.float32)
    W2 = np.asarray(W2, np.float32)
    b2 = np.asarray(b2, np.float32)

    sigma, xct, wfull, cs_list = _host_precompute(x, lambd)
    band = _make_band(sigma)
    blocks, _ = _make_blocks(band)
    wint = np.concatenate(
        [wfull[k][:, tb * TBS:(tb + 1) * TBS] for (k, tb) in blocks], axis=1
    )
    wint = np.ascontiguousarray(wint)

    W1T = _transpose_w1(W1)                     # [F*T, HID] bf16
    w1h = W1T.reshape(NCORES, 128, T * HID)     # per-core contiguous views

    nc = _build_program(band, len(blocks))

    in_maps = [
        dict(xct=xct, wint=wint, cs=cs_list[c], w1h=np.ascontiguousarray(w1h[c]))
        for c in range(NCORES)
    ]
    res = run_bass_kernel_spmd(
        nc, in_maps, list(range(NCORES)),
        trace=bool(os.environ.get("KERNEL_TRACE")),
    )
    LAST_RESULT = res

    s = np.concatenate(
        [res.results[c]["s_out"] for c in range(NCORES)], axis=1
    )                                            # [B, F, T]
    h = np.zeros((B, HID), np.float32)
    for c in range(NCORES):
        h += res.results[c]["h_out"]
    h = np.maximum(h + b1, 0.0)
    out = h @ W2.T + b2
    return out.astype(np.float32), s[:, None].astype(np.float32)
